# revision 11
# baseline (speedup 1.0000x reference)
"""Trainium2 Bass kernel for nn_DCDLayer (ragged_sequence).

Math (see reference):
    mean_f[b]  = mean of x2 rows in segment b                    [B, C]
    ha         = relu(BN(mean_f @ W1a) )  ; out_mean = relu(ha @ W2a)
    hb         = relu(BN(mean_f @ W1b) )  ; out_w    = sigmoid(relu(hb @ W2b))
    out[j]     = x2[j] * (0.5*out_w[seg j] + 0.75) + out_mean[seg j]

Sharding: 8 cores, each owns 8 whole segments (32768 contiguous rows of x2).
Per-core flow:
  phase A: PE colsum of x2 tiles -> 8 local segment means
  AllGather means [8,512] -> [64,512]  (BatchNorm couples all segments)
  MLP feature-sharded 8-ways (each core gets a 256-wide slice of MID, sliced
  on the host into its in_map), BN stats are per-feature so they stay local;
  partial second matmuls AllReduce'd ([1024,64], tiny).
  phase C: out = x2 * scale_bc[seg] + bias_bc[seg]   (2 DVE ops / tile)
"""

import sys
import numpy as np

for _p in ("/opt/trn_rl_repo",):
    if _p not in sys.path:
        sys.path.insert(0, _p)

B = 64            # segments
SEG = 4096        # rows per segment
N = B * SEG
C = 512
MID = 2048
EPS = 1e-5

NCORES = 8
B_LOC = B // NCORES          # 8 segments per core
ROWS = N // NCORES           # 32768 rows per core
FSH = MID // NCORES          # 256 features of MID per core
TPB = 4                      # 128-row tiles per DMA block (1 MiB blocks)
BLK_PER_SEG = SEG // (128 * TPB)   # 8 blocks per segment
NBLK = ROWS // (128 * TPB)   # 64 blocks per core

_CACHE = {}


def _emit(nc, tc, tile, mybir, make_identity, t):
    f32 = mybir.dt.float32
    Alu = mybir.AluOpType
    Act = mybir.ActivationFunctionType
    X = mybir.AxisListType.X
    RG = [list(range(NCORES))]

    from contextlib import ExitStack
    ctx = ExitStack()
    consts = ctx.enter_context(tc.tile_pool(name="consts", bufs=1))
    wpool = ctx.enter_context(tc.tile_pool(name="wpool", bufs=1))
    mlp = ctx.enter_context(tc.tile_pool(name="mlp", bufs=1))
    small = ctx.enter_context(tc.tile_pool(name="small", bufs=2))
    xa = ctx.enter_context(tc.tile_pool(name="xa", bufs=4))
    xcp = ctx.enter_context(tc.tile_pool(name="xcp", bufs=6))
    ocp = ctx.enter_context(tc.tile_pool(name="ocp", bufs=6))
    bcp = ctx.enter_context(tc.tile_pool(name="bcp", bufs=2))
    psA = ctx.enter_context(tc.tile_pool(name="psA", bufs=2, space="PSUM"))
    psB = ctx.enter_context(tc.tile_pool(name="psB", bufs=4, space="PSUM"))
    dram = ctx.enter_context(tc.tile_pool(name="dram", bufs=1, space="DRAM"))

    # ---- constants
    ident = consts.tile([128, 128], f32)
    make_identity(nc, ident)
    ones_col = consts.tile([128, 1], f32)
    nc.gpsimd.memset(ones_col, 1.0)
    eps_col = consts.tile([128, 1], f32)
    nc.gpsimd.memset(eps_col, EPS)
    zero_col = consts.tile([128, 1], f32)
    nc.gpsimd.memset(zero_col, 0.0)

    # ---- weights (per-core feature slices) -> SBUF
    def load_w(name, ap, p_tiles, fdim):
        out = []
        for k in range(p_tiles):
            w = wpool.tile([128, fdim], f32, tag=f"{name}{k}", name=f"{name}{k}")
            nc.sync.dma_start(w, ap[k * 128:(k + 1) * 128, :])
            out.append(w)
        return out

    w1a_sb = load_w("w1a", t["w1a"], 4, FSH)   # [512,256] -> 4x[128,256]
    w1b_sb = load_w("w1b", t["w1b"], 4, FSH)
    w2a_sb = load_w("w2a", t["w2a"], 2, C)     # [256,512] -> 2x[128,512]
    w2b_sb = load_w("w2b", t["w2b"], 2, C)

    def load_gb(name, vec):   # dram [FSH] -> SBUF [128, FSH//128] (feature on partition)
        r = mlp.tile([FSH // 128, 128], f32, tag=f"{name}r", name=f"{name}r")
        nc.sync.dma_start(r, vec.rearrange("(a b) -> a b", b=128))
        pt = psB.tile([128, FSH // 128], f32, tag="ps", name=f"{name}pt")
        nc.tensor.transpose(pt, r, ident[:FSH // 128, :FSH // 128])
        o = mlp.tile([128, FSH // 128], f32, tag=f"{name}T", name=f"{name}T")
        nc.scalar.copy(o, pt)
        return o

    gaT = load_gb("ga", t["g1a"])
    baT = load_gb("ba", t["b1a"])
    gbT = load_gb("gb", t["g1b"])
    bbT = load_gb("bb", t["b1b"])

    xv = t["x"].rearrange("(n p) c -> p n c", p=128)    # [128, 256, 512]
    ov = t["out"].rearrange("(n p) c -> p n c", p=128)

    # ---- phase A: local segment means
    agin = dram.tile([B_LOC, C], f32)
    agout = dram.tile([B, C], f32, addr_space="Shared")
    for s in range(B_LOC):
        ps = psA.tile([1, C], f32, tag="psA", name=f"psA{s}")
        for blk in range(BLK_PER_SEG):
            nb = s * BLK_PER_SEG + blk
            xt = xa.tile([128, TPB, C], f32, tag="xa", name=f"xa{nb}")
            nc.sync.dma_start(xt, xv[:, nb * TPB:(nb + 1) * TPB, :])
            for i in range(TPB):
                nc.tensor.matmul(
                    ps, lhsT=ones_col, rhs=xt[:, i, :],
                    start=(blk == 0 and i == 0),
                    stop=(blk == BLK_PER_SEG - 1 and i == TPB - 1),
                )
        msr = small.tile([1, C], f32, tag="msr", name=f"msr{s}")
        nc.scalar.mul(msr, ps, 1.0 / SEG)
        nc.sync.dma_start(agin[s:s + 1, :], msr)

    # ---- AllGather means
    nc.gpsimd.collective_compute(
        "AllGather", Alu.bypass, replica_groups=RG,
        ins=[agin.opt()], outs=[agout.opt()],
    )
    m_all = mlp.tile([B, C], f32)
    nc.sync.dma_start(m_all, agout)

    # meansT: [C(4x128), B]
    mT = []
    for k in range(4):
        pt = psB.tile([128, B], f32, tag="ps", name=f"mTp{k}")
        nc.tensor.transpose(pt, m_all[:, k * 128:(k + 1) * 128], ident[:B, :B])
        mm = mlp.tile([128, B], f32, tag=f"mT{k}", name=f"mT{k}")
        nc.scalar.copy(mm, pt)
        mT.append(mm)

    # ---- MLP branch: h1T = W1slice.T @ meansT ; BN per feature ; relu
    def branch(bid, w1_sb, gT, bT):
        haT = []
        for ml in range(FSH // 128):           # 2 local feature tiles
            ph = psB.tile([128, B], f32, tag="ps", name=f"ph{bid}{ml}")
            for k in range(4):
                nc.tensor.matmul(
                    ph, lhsT=w1_sb[k][:, ml * 128:(ml + 1) * 128], rhs=mT[k],
                    start=(k == 0), stop=(k == 3),
                )
            h = mlp.tile([128, B], f32, tag=f"h{bid}{ml}", name=f"h{bid}{ml}")
            nc.scalar.copy(h, ph)
            s1 = small.tile([128, 1], f32, tag="s1", name=f"s1{bid}{ml}")
            nc.vector.tensor_reduce(s1, h, axis=X, op=Alu.add)
            sq = small.tile([128, B], f32, tag="sq", name=f"sq{bid}{ml}")
            nc.scalar.activation(sq, h, Act.Square, bias=zero_col)
            s2 = small.tile([128, 1], f32, tag="s2", name=f"s2{bid}{ml}")
            nc.vector.tensor_reduce(s2, sq, axis=X, op=Alu.add)
            mu = small.tile([128, 1], f32, tag="mu", name=f"mu{bid}{ml}")
            nc.scalar.mul(mu, s1, 1.0 / B)
            ex2 = small.tile([128, 1], f32, tag="ex2", name=f"ex2{bid}{ml}")
            nc.scalar.mul(ex2, s2, 1.0 / B)
            mu2 = small.tile([128, 1], f32, tag="mu2", name=f"mu2{bid}{ml}")
            nc.scalar.activation(mu2, mu, Act.Square, bias=zero_col)
            var = small.tile([128, 1], f32, tag="var", name=f"var{bid}{ml}")
            nc.vector.tensor_sub(var, ex2, mu2)
            std = small.tile([128, 1], f32, tag="std", name=f"std{bid}{ml}")
            nc.scalar.activation(std, var, Act.Sqrt, bias=eps_col)
            istd = small.tile([128, 1], f32, tag="istd", name=f"istd{bid}{ml}")
            nc.vector.reciprocal(istd, std)
            sc = small.tile([128, 1], f32, tag="sc", name=f"sc{bid}{ml}")
            nc.vector.tensor_mul(sc, gT[:, ml:ml + 1], istd)
            t1 = small.tile([128, 1], f32, tag="t1", name=f"t1{bid}{ml}")
            nc.vector.tensor_mul(t1, mu, sc)
            bi = small.tile([128, 1], f32, tag="bi", name=f"bi{bid}{ml}")
            nc.vector.tensor_sub(bi, bT[:, ml:ml + 1], t1)
            ha = mlp.tile([128, B], f32, tag=f"ha{bid}{ml}", name=f"ha{bid}{ml}")
            nc.scalar.activation(ha, h, Act.Relu, bias=bi, scale=sc)
            haT.append(ha)
        return haT

    haTa = branch("a", w1a_sb, gaT, baT)
    haTb = branch("b", w1b_sb, gbT, bbT)

    # ---- partial second matmuls -> AllReduce
    arin = dram.tile([2 * C, B], f32)
    arout = dram.tile([2 * C, B], f32, addr_space="Shared")
    for bi_, (w2_sb, haT) in enumerate([(w2a_sb, haTa), (w2b_sb, haTb)]):
        for j in range(4):
            po = psB.tile([128, B], f32, tag="ps", name=f"po{bi_}{j}")
            for ml in range(FSH // 128):
                nc.tensor.matmul(
                    po, lhsT=w2_sb[ml][:, j * 128:(j + 1) * 128], rhs=haT[ml],
                    start=(ml == 0), stop=(ml == FSH // 128 - 1),
                )
            pos = small.tile([128, B], f32, tag="pos", name=f"pos{bi_}{j}")
            nc.scalar.copy(pos, po)
            nc.sync.dma_start(arin[bi_ * C + j * 128: bi_ * C + (j + 1) * 128, :], pos)
    nc.gpsimd.collective_compute(
        "AllReduce", Alu.add, replica_groups=RG,
        ins=[arin.opt()], outs=[arout.opt()],
    )

    # ---- post-AR: nonlinearities, transpose to row layout
    rowsB = mlp.tile([B, C], f32)   # bias rows  (out_mean)
    rowsS = mlp.tile([B, C], f32)   # scale rows (0.5*out_w + 0.75)
    for j in range(4):
        ta = small.tile([128, B], f32, tag="post_a", name=f"ta{j}")
        nc.sync.dma_start(ta, arout[j * 128:(j + 1) * 128, :])
        oa = small.tile([128, B], f32, tag="post_oa", name=f"oa{j}")
        nc.scalar.activation(oa, ta, Act.Relu, bias=zero_col)
        pt = psB.tile([B, 128], f32, tag="ps", name=f"pta{j}")
        nc.tensor.transpose(pt, oa, ident)
        nc.scalar.copy(rowsB[:, j * 128:(j + 1) * 128], pt)

        tb = small.tile([128, B], f32, tag="post_b", name=f"tb{j}")
        nc.sync.dma_start(tb, arout[C + j * 128: C + (j + 1) * 128, :])
        ob = small.tile([128, B], f32, tag="post_ob", name=f"ob{j}")
        nc.scalar.activation(ob, tb, Act.Relu, bias=zero_col)
        ob2 = small.tile([128, B], f32, tag="post_ob2", name=f"ob2{j}")
        nc.scalar.activation(ob2, ob, Act.Sigmoid, bias=zero_col)
        ob3 = small.tile([128, B], f32, tag="post_ob3", name=f"ob3{j}")
        nc.scalar.activation(ob3, ob2, Act.Copy, bias=0.75, scale=0.5)
        pt2 = psB.tile([B, 128], f32, tag="ps", name=f"ptb{j}")
        nc.tensor.transpose(pt2, ob3, ident)
        nc.scalar.copy(rowsS[:, j * 128:(j + 1) * 128], pt2)

    # ---- per-core replicated one-hot selector [64, 8, 128]:
    # sel_all[:, s, :].T @ rows = broadcast of row (8c+s) of rows to 128 partitions
    sel_all = mlp.tile([B, B_LOC, 128], f32)
    selv = t["sel"].rearrange("(s k) p -> k s p", s=B_LOC)
    nc.sync.dma_start(sel_all, selv)

    # ---- phase C: out = x2 * scale_bc + bias_bc
    for s in range(B_LOC):
        pbs = psB.tile([128, C], f32, tag="ps", name=f"pbs{s}")
        nc.tensor.matmul(pbs, lhsT=sel_all[:, s, :], rhs=rowsS,
                         start=True, stop=True)
        sbc = bcp.tile([128, C], f32, tag="sbc", name=f"sbc{s}")
        nc.scalar.copy(sbc, pbs)
        pbb = psB.tile([128, C], f32, tag="ps", name=f"pbb{s}")
        nc.tensor.matmul(pbb, lhsT=sel_all[:, s, :], rhs=rowsB,
                         start=True, stop=True)
        bbc = bcp.tile([128, C], f32, tag="bbc", name=f"bbc{s}")
        nc.scalar.copy(bbc, pbb)
        for blk in range(BLK_PER_SEG):
            nb = s * BLK_PER_SEG + blk
            xt = xcp.tile([128, TPB, C], f32, tag="xc", name=f"xc{nb}")
            nc.sync.dma_start(xt, xv[:, nb * TPB:(nb + 1) * TPB, :])
            ot = ocp.tile([128, TPB, C], f32, tag="oc", name=f"oc{nb}")
            for i in range(TPB):
                nc.vector.tensor_mul(ot[:, i, :], xt[:, i, :], sbc)
                nc.vector.tensor_add(ot[:, i, :], ot[:, i, :], bbc)
            nc.sync.dma_start(ov[:, nb * TPB:(nb + 1) * TPB, :], ot)

    ctx.close()


def _build():
    if "nc" in _CACHE:
        return _CACHE["nc"]
    import concourse.bacc as bacc
    import concourse.tile as tile
    from concourse import mybir
    from concourse.masks import make_identity

    f32 = mybir.dt.float32
    nc = bacc.Bacc("TRN2", target_bir_lowering=False, debug=False,
                   enable_asserts=False, num_devices=NCORES)
    t = {
        "x": nc.dram_tensor("x", [ROWS, C], f32, kind="ExternalInput").ap(),
        "w1a": nc.dram_tensor("w1a", [C, FSH], f32, kind="ExternalInput").ap(),
        "w2a": nc.dram_tensor("w2a", [FSH, C], f32, kind="ExternalInput").ap(),
        "w1b": nc.dram_tensor("w1b", [C, FSH], f32, kind="ExternalInput").ap(),
        "w2b": nc.dram_tensor("w2b", [FSH, C], f32, kind="ExternalInput").ap(),
        "g1a": nc.dram_tensor("g1a", [FSH], f32, kind="ExternalInput").ap(),
        "b1a": nc.dram_tensor("b1a", [FSH], f32, kind="ExternalInput").ap(),
        "g1b": nc.dram_tensor("g1b", [FSH], f32, kind="ExternalInput").ap(),
        "b1b": nc.dram_tensor("b1b", [FSH], f32, kind="ExternalInput").ap(),
        "sel": nc.dram_tensor("sel", [B_LOC * B, 128], f32, kind="ExternalInput").ap(),
        "out": nc.dram_tensor("out", [ROWS, C], f32, kind="ExternalOutput").ap(),
    }
    with tile.TileContext(nc) as tc:
        _emit(nc, tc, tile, mybir, make_identity, t)
    nc.compile()
    _CACHE["nc"] = nc
    return nc


def _make_in_maps(x2, W1a, g1a, b1a, W2a, W1b, g1b, b1b, W2b):
    in_maps = []
    for c in range(NCORES):
        f0, f1 = c * FSH, (c + 1) * FSH
        sel = np.zeros((B_LOC, B, 128), np.float32)
        sel[np.arange(B_LOC), c * B_LOC + np.arange(B_LOC), :] = 1.0
        sel = sel.reshape(B_LOC * B, 128)
        in_maps.append({
            "x": np.ascontiguousarray(x2[c * ROWS:(c + 1) * ROWS]),
            "w1a": np.ascontiguousarray(W1a[:, f0:f1]),
            "w2a": np.ascontiguousarray(W2a[f0:f1, :]),
            "w1b": np.ascontiguousarray(W1b[:, f0:f1]),
            "w2b": np.ascontiguousarray(W2b[f0:f1, :]),
            "g1a": np.ascontiguousarray(g1a[f0:f1]),
            "b1a": np.ascontiguousarray(b1a[f0:f1]),
            "g1b": np.ascontiguousarray(g1b[f0:f1]),
            "b1b": np.ascontiguousarray(b1b[f0:f1]),
            "sel": sel,
        })
    return in_maps


def _numpy_fallback(x2, npoint, W1a, g1a, b1a, W2a, W1b, g1b, b1b, W2b):
    n = x2.shape[0]
    b = npoint.shape[0]
    cum = np.cumsum(npoint)
    seg = np.searchsorted(cum, np.arange(n), side="right")
    counts = npoint.astype(x2.dtype)
    sums = np.zeros((b, x2.shape[1]), x2.dtype)
    np.add.at(sums, seg, x2)
    mean_f = sums / counts[:, None]

    def bn(h, g, bb):
        m = h.mean(0)
        v = h.var(0)
        return (h - m) / np.sqrt(v + EPS) * g + bb

    ha = np.maximum(bn(mean_f @ W1a, g1a, b1a), 0)
    out_mean = np.maximum(ha @ W2a, 0)
    hb = np.maximum(bn(mean_f @ W1b, g1b, b1b), 0)
    zw = np.maximum(hb @ W2b, 0)
    out_w = 1.0 / (1.0 + np.exp(-zw))
    return out_w[seg] * x2 * 0.5 + x2 * 0.75 + out_mean[seg]


def run_on_device(inputs, trace=False, **kwargs):
    """Returns (full_output, BassKernelResults)."""
    from concourse import bass_utils
    x2 = np.asarray(inputs["x2"], np.float32)
    args = {k: np.asarray(inputs[k], np.float32)
            for k in ("W1a", "g1a", "b1a", "W2a", "W1b", "g1b", "b1b", "W2b")}
    nc = _build()
    in_maps = _make_in_maps(x2, args["W1a"], args["g1a"], args["b1a"],
                            args["W2a"], args["W1b"], args["g1b"],
                            args["b1b"], args["W2b"])
    res = bass_utils.run_bass_kernel_spmd(
        nc, in_maps, core_ids=list(range(NCORES)), trace=trace, **kwargs)
    out = np.concatenate([res.results[c]["out"] for c in range(NCORES)], axis=0)
    return out, res


def bench_device(inputs, iters=10, warmup=2):
    """Time the sharded NEFF execution with inputs pre-staged on device.

    Returns (times_sec_list, output). Mirrors bass2jax.run_bass_via_pjrt's
    multi-core path but without donation so the callable can be re-invoked.
    """
    import time
    import jax
    from jax.experimental.shard_map import shard_map
    from jax.sharding import Mesh, NamedSharding, PartitionSpec
    from concourse import bass2jax, mybir

    nc = _build()
    x2 = np.asarray(inputs["x2"], np.float32)
    args = {k: np.asarray(inputs[k], np.float32)
            for k in ("W1a", "g1a", "b1a", "W2a", "W1b", "g1b", "b1b", "W2b")}
    in_maps = _make_in_maps(x2, args["W1a"], args["g1a"], args["b1a"],
                            args["W2a"], args["W1b"], args["g1b"],
                            args["b1b"], args["W2b"])

    bass2jax.install_neuronx_cc_hook()
    partition_name = (nc.partition_id_tensor.name
                      if nc.partition_id_tensor else None)
    in_names, out_names, out_avals, zero_outs = [], [], [], []
    for alloc in nc.m.functions[0].allocations:
        if not isinstance(alloc, mybir.MemoryLocationSet):
            continue
        name = alloc.memorylocations[0].name
        if alloc.kind == "ExternalInput":
            if name != partition_name:
                in_names.append(name)
        elif alloc.kind == "ExternalOutput":
            shape = tuple(alloc.tensor_shape)
            dtype = mybir.dt.np(alloc.dtype)
            out_names.append(name)
            out_avals.append(jax.core.ShapedArray(shape, dtype))
            zero_outs.append(np.zeros(shape, dtype))
    n_params = len(in_names)
    all_in_names = list(in_names) + list(out_names)
    if partition_name is not None:
        all_in_names.append(partition_name)

    def _body(*a):
        operands = list(a)
        if partition_name is not None:
            operands.append(bass2jax.partition_id_tensor())
        outs = bass2jax._bass_exec_p.bind(
            *operands,
            out_avals=tuple(out_avals),
            in_names=tuple(all_in_names),
            out_names=tuple(out_names),
            lowering_input_output_aliases=(),
            sim_require_finite=True,
            sim_require_nnan=True,
            nc=nc,
        )
        return tuple(outs)

    devices = jax.devices()[:NCORES]
    mesh = Mesh(np.asarray(devices), ("core",))
    spec = PartitionSpec("core")
    n_outs = len(out_names)
    fn = jax.jit(
        shard_map(_body, mesh=mesh,
                  in_specs=(spec,) * (n_params + n_outs),
                  out_specs=(spec,) * n_outs, check_rep=False),
        keep_unused=True,
    )
    sharding = NamedSharding(mesh, spec)
    concat_in = [
        jax.device_put(
            np.concatenate([np.asarray(in_maps[c][nm]) for c in range(NCORES)],
                           axis=0), sharding)
        for nm in in_names
    ]
    concat_zero = [
        jax.device_put(np.zeros((NCORES * z.shape[0], *z.shape[1:]), z.dtype),
                       sharding)
        for z in zero_outs
    ]
    for _ in range(warmup):
        r = fn(*concat_in, *concat_zero)
        jax.block_until_ready(r)
    times = []
    for _ in range(iters):
        t0 = time.perf_counter()
        r = fn(*concat_in, *concat_zero)
        jax.block_until_ready(r)
        times.append(time.perf_counter() - t0)
    out = np.asarray(r[0]).reshape(NCORES, ROWS, C).reshape(N, C)
    return times, out


def kernel(**inputs):
    x2 = np.asarray(inputs["x2"], np.float32)
    npoint = np.asarray(inputs["npoint"])
    if (x2.shape != (N, C) or npoint.shape != (B,)
            or not np.all(npoint == SEG)):
        return _numpy_fallback(
            x2, npoint,
            *[np.asarray(inputs[k], np.float32)
              for k in ("W1a", "g1a", "b1a", "W2a", "W1b", "g1b", "b1b", "W2b")],
        ).astype(np.float32)
    out, _ = run_on_device(inputs)
    return out


# revision 23
# speedup vs baseline: 129.3467x; 129.3467x over previous
"""Trainium2 Bass kernel for nn_DCDLayer (ragged_sequence).

Math (see reference):
    mean_f[b]  = mean of x2 rows in segment b                    [B, C]
    ha         = relu(BN(mean_f @ W1a) )  ; out_mean = relu(ha @ W2a)
    hb         = relu(BN(mean_f @ W1b) )  ; out_w    = sigmoid(relu(hb @ W2b))
    out[j]     = x2[j] * (0.5*out_w[seg j] + 0.75) + out_mean[seg j]

Sharding: 8 cores, each owns 8 whole segments (32768 contiguous rows of x2).
Per-core flow:
  phase A: PE colsum of x2 tiles -> 8 local segment means
  AllGather means [8,512] -> [64,512]  (BatchNorm couples all segments)
  MLP feature-sharded 8-ways (each core gets a 256-wide slice of MID, sliced
  on the host into its in_map), BN stats are per-feature so they stay local;
  partial second matmuls AllReduce'd ([1024,64], tiny).
  phase C: out = x2 * scale_bc[seg] + bias_bc[seg]   (2 DVE ops / tile)
"""

import sys
import numpy as np

for _p in ("/opt/trn_rl_repo",):
    if _p not in sys.path:
        sys.path.insert(0, _p)

B = 64            # segments
SEG = 4096        # rows per segment
N = B * SEG
C = 512
MID = 2048
EPS = 1e-5

NCORES = 8
B_LOC = B // NCORES          # 8 segments per core
ROWS = N // NCORES           # 32768 rows per core
FSH = MID // NCORES          # 256 features of MID per core
TPB = 4                      # 128-row tiles per DMA block (1 MiB blocks)
BLK_PER_SEG = SEG // (128 * TPB)   # 8 blocks per segment
NBLK = ROWS // (128 * TPB)   # 64 blocks per core

_CACHE = {}


def _emit(nc, tc, tile, mybir, make_identity, t, collectives=True):
    f32 = mybir.dt.float32
    f32r = mybir.dt.float32r
    Alu = mybir.AluOpType
    Act = mybir.ActivationFunctionType
    X = mybir.AxisListType.X
    RG = [list(range(NCORES))]

    from contextlib import ExitStack
    ctx = ExitStack()
    consts = ctx.enter_context(tc.tile_pool(name="consts", bufs=1))
    wpool = ctx.enter_context(tc.tile_pool(name="wpool", bufs=1))
    mlp = ctx.enter_context(tc.tile_pool(name="mlp", bufs=1))
    small = ctx.enter_context(tc.tile_pool(name="small", bufs=2))
    xa = ctx.enter_context(tc.tile_pool(name="xa", bufs=3))
    xsp = ctx.enter_context(tc.tile_pool(name="xsp", bufs=2))
    accp = ctx.enter_context(tc.tile_pool(name="accp", bufs=2))
    xcp = ctx.enter_context(tc.tile_pool(name="xcp", bufs=7))
    resp = ctx.enter_context(tc.tile_pool(name="resp", bufs=8))
    bcp = ctx.enter_context(tc.tile_pool(name="bcp", bufs=2))
    psA = ctx.enter_context(tc.tile_pool(name="psA", bufs=3, space="PSUM"))
    psB = ctx.enter_context(tc.tile_pool(name="psB", bufs=4, space="PSUM"))
    dram = ctx.enter_context(tc.tile_pool(name="dram", bufs=1, space="DRAM"))

    # ---- constants
    ident = consts.tile([128, 128], f32)
    make_identity(nc, ident)
    ones_col = consts.tile([128, 1], f32)
    nc.gpsimd.memset(ones_col, 1.0)
    eps_col = consts.tile([128, 1], f32)
    nc.gpsimd.memset(eps_col, EPS)
    zero_col = consts.tile([128, 1], f32)
    nc.gpsimd.memset(zero_col, 0.0)

    # ---- weights (per-core feature slices) -> SBUF
    def load_w(name, ap, p_tiles, fdim):
        out = []
        for k in range(p_tiles):
            w = wpool.tile([128, fdim], f32, tag=f"{name}{k}", name=f"{name}{k}")
            nc.sync.dma_start(w, ap[k * 128:(k + 1) * 128, :])
            out.append(w)
        return out

    w1a_sb = load_w("w1a", t["w1a"], 4, FSH)   # [512,256] -> 4x[128,256]
    w1b_sb = load_w("w1b", t["w1b"], 4, FSH)
    w2a_sb = load_w("w2a", t["w2a"], 2, C)     # [256,512] -> 2x[128,512]
    w2b_sb = load_w("w2b", t["w2b"], 2, C)

    def load_gb(name, vec):   # dram [FSH] -> SBUF [128, FSH//128] (feature on partition)
        r = mlp.tile([FSH // 128, 128], f32, tag=f"{name}r", name=f"{name}r")
        nc.sync.dma_start(r, vec.rearrange("(a b) -> a b", b=128))
        pt = psB.tile([128, FSH // 128], f32, tag="ps", name=f"{name}pt")
        nc.tensor.transpose(pt, r, ident[:FSH // 128, :FSH // 128])
        o = mlp.tile([128, FSH // 128], f32, tag=f"{name}T", name=f"{name}T")
        nc.scalar.copy(o, pt)
        return o

    gaT = load_gb("ga", t["g1a"])
    baT = load_gb("ba", t["b1a"])
    gbT = load_gb("gb", t["g1b"])
    bbT = load_gb("bb", t["b1b"])

    xv = t["x"].rearrange("(n p) c -> p n c", p=128)    # [128, 256, 512]
    ov = t["out"].rearrange("(n p) c -> p n c", p=128)

    # ---- phase A: local segment means
    # segment RES_SEG's blocks stay resident in SBUF (xcp pool) and are
    # combined first in phase C without a re-load.
    RES_SEG = B_LOC - 1
    res_tiles = {}
    last_a_load = [None]
    agin = dram.tile([B_LOC, C], f32)
    agout = dram.tile([B, C], f32,
                      addr_space="Shared" if collectives else "Local")
    for s in [RES_SEG] + [s for s in range(B_LOC) if s != RES_SEG]:
        acc = accp.tile([128, C], f32, tag="acc", name=f"acc{s}")
        for blk in range(BLK_PER_SEG):
            nb = s * BLK_PER_SEG + blk
            if s == RES_SEG:
                xt = resp.tile([128, TPB, C], f32, tag="xr", name=f"xres{blk}")
                res_tiles[blk] = xt
            else:
                xt = xa.tile([128, TPB, C], f32, tag="xa", name=f"xa{nb}")
            last_a_load[0] = nc.sync.dma_start(
                xt, xv[:, nb * TPB:(nb + 1) * TPB, :])
            # pre-reduce the 4 tiles on DVE (idle in phase A); POOL (also
            # idle) accumulates blocks into a per-segment [128, C] partial;
            # PE then does ONE fp32 colsum matmul per segment.
            xs = xsp.tile([128, C], f32, tag="xs", name=f"xs{nb}")
            nc.vector.tensor_add(xs, xt[:, 0, :], xt[:, 1, :])
            nc.vector.tensor_add(xs, xs, xt[:, 2, :])
            nc.vector.tensor_add(xs, xs, xt[:, 3, :])
            if blk == 0:
                nc.gpsimd.tensor_copy(acc, xs)
            else:
                nc.gpsimd.tensor_add(acc, acc, xs)
        ps = psA.tile([1, C], f32, tag="psA", name=f"psA{s}")
        nc.tensor.matmul(ps, lhsT=ones_col, rhs=acc, start=True, stop=True)
        msr = small.tile([1, C], f32, tag="msr", name=f"msr{s}")
        nc.scalar.mul(msr, ps, 1.0 / SEG)
        nc.sync.dma_start(agin[s:s + 1, :], msr)

    # ---- AllGather means
    if collectives:
        nc.gpsimd.collective_compute(
            "AllGather", Alu.bypass, replica_groups=RG,
            ins=[agin.opt()], outs=[agout.opt()],
        )
    else:
        nc.sync.dma_start(agout[:B_LOC, :], agin)
    m_all = mlp.tile([B, C], f32)
    nc.sync.dma_start(m_all, agout)

    # meansT: [C(4x128), B]
    mT = []
    for k in range(4):
        pt = psB.tile([128, B], f32, tag="ps", name=f"mTp{k}")
        nc.tensor.transpose(pt, m_all[:, k * 128:(k + 1) * 128], ident[:B, :B])
        mm = mlp.tile([128, B], f32, tag=f"mT{k}", name=f"mT{k}")
        nc.scalar.copy(mm, pt)
        mT.append(mm)

    # ---- MLP branch: h1T = W1slice.T @ meansT ; BN per feature ; relu
    def branch(bid, w1_sb, gT, bT):
        haT = []
        for ml in range(FSH // 128):           # 2 local feature tiles
            ph = psB.tile([128, B], f32, tag="ps", name=f"ph{bid}{ml}")
            for k in range(4):
                nc.tensor.matmul(
                    ph, lhsT=w1_sb[k][:, ml * 128:(ml + 1) * 128], rhs=mT[k],
                    start=(k == 0), stop=(k == 3),
                )
            h = mlp.tile([128, B], f32, tag=f"h{bid}{ml}", name=f"h{bid}{ml}")
            nc.scalar.copy(h, ph)
            s1 = small.tile([128, 1], f32, tag="s1", name=f"s1{bid}{ml}")
            nc.vector.tensor_reduce(s1, h, axis=X, op=Alu.add)
            sq = small.tile([128, B], f32, tag="sq", name=f"sq{bid}{ml}")
            nc.scalar.activation(sq, h, Act.Square, bias=zero_col)
            s2 = small.tile([128, 1], f32, tag="s2", name=f"s2{bid}{ml}")
            nc.vector.tensor_reduce(s2, sq, axis=X, op=Alu.add)
            mu = small.tile([128, 1], f32, tag="mu", name=f"mu{bid}{ml}")
            nc.scalar.mul(mu, s1, 1.0 / B)
            ex2 = small.tile([128, 1], f32, tag="ex2", name=f"ex2{bid}{ml}")
            nc.scalar.mul(ex2, s2, 1.0 / B)
            mu2 = small.tile([128, 1], f32, tag="mu2", name=f"mu2{bid}{ml}")
            nc.scalar.activation(mu2, mu, Act.Square, bias=zero_col)
            var = small.tile([128, 1], f32, tag="var", name=f"var{bid}{ml}")
            nc.vector.tensor_sub(var, ex2, mu2)
            std = small.tile([128, 1], f32, tag="std", name=f"std{bid}{ml}")
            nc.scalar.activation(std, var, Act.Sqrt, bias=eps_col)
            istd = small.tile([128, 1], f32, tag="istd", name=f"istd{bid}{ml}")
            nc.vector.reciprocal(istd, std)
            sc = small.tile([128, 1], f32, tag="sc", name=f"sc{bid}{ml}")
            nc.vector.tensor_mul(sc, gT[:, ml:ml + 1], istd)
            t1 = small.tile([128, 1], f32, tag="t1", name=f"t1{bid}{ml}")
            nc.vector.tensor_mul(t1, mu, sc)
            bi = small.tile([128, 1], f32, tag="bi", name=f"bi{bid}{ml}")
            nc.vector.tensor_sub(bi, bT[:, ml:ml + 1], t1)
            ha = mlp.tile([128, B], f32, tag=f"ha{bid}{ml}", name=f"ha{bid}{ml}")
            nc.scalar.activation(ha, h, Act.Relu, bias=bi, scale=sc)
            haT.append(ha)
        return haT

    haTa = branch("a", w1a_sb, gaT, baT)
    haTb = branch("b", w1b_sb, gbT, bbT)

    # ---- partial second matmuls -> AllReduce (staged as one batched DMA)
    arin = dram.tile([2 * C, B], f32)
    arout = dram.tile([2 * C, B], f32,
                      addr_space="Shared" if collectives else "Local")
    pos_all = mlp.tile([128, 8, B], f32)
    for bi_, (w2_sb, haT) in enumerate([(w2a_sb, haTa), (w2b_sb, haTb)]):
        for j in range(4):
            po = psB.tile([128, B], f32, tag="ps", name=f"po{bi_}{j}")
            for ml in range(FSH // 128):
                nc.tensor.matmul(
                    po, lhsT=w2_sb[ml][:, j * 128:(j + 1) * 128], rhs=haT[ml],
                    start=(ml == 0), stop=(ml == FSH // 128 - 1),
                )
            nc.scalar.copy(pos_all[:, bi_ * 4 + j, :], po)
    nc.sync.dma_start(arin.rearrange("(g p) b -> p g b", p=128), pos_all)
    if collectives:
        nc.gpsimd.collective_compute(
            "AllReduce", Alu.add, replica_groups=RG,
            ins=[arin.opt()], outs=[arout.opt()],
        )
    else:
        nc.sync.dma_start(arout[:, :], arin)

    # ---- post-AR: nonlinearities, transpose to row layout
    rowsB = mlp.tile([B, C], f32)   # bias rows  (out_mean)
    rowsS = mlp.tile([B, C], f32)   # scale rows (0.5*out_w + 0.75)
    post_all = mlp.tile([128, 8, B], f32)
    nc.sync.dma_start(post_all, arout.rearrange("(g p) b -> p g b", p=128))
    for j in range(4):
        oa = small.tile([128, B], f32, tag="post_oa", name=f"oa{j}")
        nc.scalar.activation(oa, post_all[:, j, :], Act.Relu, bias=zero_col)
        pt = psB.tile([B, 128], f32, tag="ps", name=f"pta{j}")
        nc.tensor.transpose(pt, oa, ident)
        nc.scalar.copy(rowsB[:, j * 128:(j + 1) * 128], pt)

        ob = small.tile([128, B], f32, tag="post_ob", name=f"ob{j}")
        nc.scalar.activation(ob, post_all[:, 4 + j, :], Act.Relu, bias=zero_col)
        ob2 = small.tile([128, B], f32, tag="post_ob2", name=f"ob2{j}")
        nc.scalar.activation(ob2, ob, Act.Sigmoid, bias=zero_col)
        ob3 = small.tile([128, B], f32, tag="post_ob3", name=f"ob3{j}")
        nc.scalar.activation(ob3, ob2, Act.Copy, bias=0.75, scale=0.5)
        pt2 = psB.tile([B, 128], f32, tag="ps", name=f"ptb{j}")
        nc.tensor.transpose(pt2, ob3, ident)
        nc.vector.tensor_copy(rowsS[:, j * 128:(j + 1) * 128], pt2)

    # ---- per-core replicated one-hot selector [64, 8, 128]:
    # sel_all[:, s, :].T @ rows = broadcast of row (8c+s) of rows to 128 partitions
    sel_all = mlp.tile([B, B_LOC, 128], f32)
    selv = t["sel"].rearrange("(s k) p -> k s p", s=B_LOC)
    nc.sync.dma_start(sel_all, selv)

    # ---- phase C: out = x2 * scale_bc + bias_bc (resident segment first)
    n_deferred = [0]
    for s in [RES_SEG] + [s for s in range(B_LOC) if s != RES_SEG]:
        pbs = psB.tile([128, C], f32, tag="ps", name=f"pbs{s}")
        nc.tensor.matmul(pbs, lhsT=sel_all[:, s, :], rhs=rowsS,
                         start=True, stop=True)
        sbc = bcp.tile([128, C], f32, tag="sbc", name=f"sbc{s}")
        nc.scalar.copy(sbc, pbs)  # ACT
        pbb = psB.tile([128, C], f32, tag="ps", name=f"pbb{s}")
        nc.tensor.matmul(pbb, lhsT=sel_all[:, s, :], rhs=rowsB,
                         start=True, stop=True)
        bbc = bcp.tile([128, C], f32, tag="bbc", name=f"bbc{s}")
        nc.vector.tensor_copy(bbc, pbb)  # DVE (split engines)
        sbc_b = sbc[:, None, :].broadcast_to([128, TPB, C])
        bbc_b = bbc[:, None, :].broadcast_to([128, TPB, C])
        for blk in range(BLK_PER_SEG):
            nb = s * BLK_PER_SEG + blk
            if s == RES_SEG:
                xt = res_tiles[blk]
            else:
                xt = xcp.tile([128, TPB, C], f32, tag="xc", name=f"xc{nb}")
                ld = nc.sync.dma_start(xt, xv[:, nb * TPB:(nb + 1) * TPB, :])
                if n_deferred[0] < 8 and last_a_load[0] is not None:
                    # keep phase-A loads (the means critical path) ahead of
                    # phase-C prefetch; prefetch then fills the MLP gap
                    tile.add_dep_helper(
                        ld.ins, last_a_load[0].ins, sync=True,
                        reason="defer phase-C prefetch behind phase-A loads")
                    n_deferred[0] += 1
            nc.vector.tensor_mul(xt, xt, sbc_b)
            nc.vector.tensor_add(xt, xt, bbc_b)
            nc.sync.dma_start(ov[:, nb * TPB:(nb + 1) * TPB, :], xt)

    ctx.close()


def _build(num_devices=NCORES, collectives=True):
    key = ("nc", num_devices, collectives)
    if key in _CACHE:
        return _CACHE[key]
    import concourse.bacc as bacc
    import concourse.tile as tile
    from concourse import mybir
    from concourse.masks import make_identity

    f32 = mybir.dt.float32
    nc = bacc.Bacc("TRN2", target_bir_lowering=False, debug=False,
                   enable_asserts=False, num_devices=num_devices)
    t = {
        "x": nc.dram_tensor("x", [ROWS, C], f32, kind="ExternalInput").ap(),
        "w1a": nc.dram_tensor("w1a", [C, FSH], f32, kind="ExternalInput").ap(),
        "w2a": nc.dram_tensor("w2a", [FSH, C], f32, kind="ExternalInput").ap(),
        "w1b": nc.dram_tensor("w1b", [C, FSH], f32, kind="ExternalInput").ap(),
        "w2b": nc.dram_tensor("w2b", [FSH, C], f32, kind="ExternalInput").ap(),
        "g1a": nc.dram_tensor("g1a", [FSH], f32, kind="ExternalInput").ap(),
        "b1a": nc.dram_tensor("b1a", [FSH], f32, kind="ExternalInput").ap(),
        "g1b": nc.dram_tensor("g1b", [FSH], f32, kind="ExternalInput").ap(),
        "b1b": nc.dram_tensor("b1b", [FSH], f32, kind="ExternalInput").ap(),
        "sel": nc.dram_tensor("sel", [B_LOC * B, 128], f32, kind="ExternalInput").ap(),
        "out": nc.dram_tensor("out", [ROWS, C], f32, kind="ExternalOutput").ap(),
    }
    with tile.TileContext(nc) as tc:
        _emit(nc, tc, tile, mybir, make_identity, t, collectives=collectives)
    nc.compile()
    _CACHE[key] = nc
    return nc


def _make_in_maps(x2, W1a, g1a, b1a, W2a, W1b, g1b, b1b, W2b):
    in_maps = []
    for c in range(NCORES):
        f0, f1 = c * FSH, (c + 1) * FSH
        sel = np.zeros((B_LOC, B, 128), np.float32)
        sel[np.arange(B_LOC), c * B_LOC + np.arange(B_LOC), :] = 1.0
        sel = sel.reshape(B_LOC * B, 128)
        in_maps.append({
            "x": np.ascontiguousarray(x2[c * ROWS:(c + 1) * ROWS]),
            "w1a": np.ascontiguousarray(W1a[:, f0:f1]),
            "w2a": np.ascontiguousarray(W2a[f0:f1, :]),
            "w1b": np.ascontiguousarray(W1b[:, f0:f1]),
            "w2b": np.ascontiguousarray(W2b[f0:f1, :]),
            "g1a": np.ascontiguousarray(g1a[f0:f1]),
            "b1a": np.ascontiguousarray(b1a[f0:f1]),
            "g1b": np.ascontiguousarray(g1b[f0:f1]),
            "b1b": np.ascontiguousarray(b1b[f0:f1]),
            "sel": sel,
        })
    return in_maps


def _numpy_fallback(x2, npoint, W1a, g1a, b1a, W2a, W1b, g1b, b1b, W2b):
    n = x2.shape[0]
    b = npoint.shape[0]
    cum = np.cumsum(npoint)
    seg = np.searchsorted(cum, np.arange(n), side="right")
    counts = npoint.astype(x2.dtype)
    sums = np.zeros((b, x2.shape[1]), x2.dtype)
    np.add.at(sums, seg, x2)
    mean_f = sums / counts[:, None]

    def bn(h, g, bb):
        m = h.mean(0)
        v = h.var(0)
        return (h - m) / np.sqrt(v + EPS) * g + bb

    ha = np.maximum(bn(mean_f @ W1a, g1a, b1a), 0)
    out_mean = np.maximum(ha @ W2a, 0)
    hb = np.maximum(bn(mean_f @ W1b, g1b, b1b), 0)
    zw = np.maximum(hb @ W2b, 0)
    out_w = 1.0 / (1.0 + np.exp(-zw))
    return out_w[seg] * x2 * 0.5 + x2 * 0.75 + out_mean[seg]


def run_on_device(inputs, trace=False, **kwargs):
    """Returns (full_output, BassKernelResults)."""
    from concourse import bass_utils
    x2 = np.asarray(inputs["x2"], np.float32)
    args = {k: np.asarray(inputs[k], np.float32)
            for k in ("W1a", "g1a", "b1a", "W2a", "W1b", "g1b", "b1b", "W2b")}
    nc = _build()
    in_maps = _make_in_maps(x2, args["W1a"], args["g1a"], args["b1a"],
                            args["W2a"], args["W1b"], args["g1b"],
                            args["b1b"], args["W2b"])
    res = bass_utils.run_bass_kernel_spmd(
        nc, in_maps, core_ids=list(range(NCORES)), trace=trace, **kwargs)
    out = np.concatenate([res.results[c]["out"] for c in range(NCORES)], axis=0)
    return out, res


def bench_device(inputs, iters=10, warmup=2, chain=1):
    """Time the sharded NEFF execution with inputs pre-staged on device.

    chain=N runs the kernel N times back-to-back inside one dispatch (each
    call's output feeds the next call's x), so per-call device time can be
    separated from the ~80ms axon dispatch floor via (T(N)-T(1))/(N-1).

    Returns (times_sec_list, output). Mirrors bass2jax.run_bass_via_pjrt's
    multi-core path but without donation so the callable can be re-invoked.
    """
    import time
    import jax
    from jax.experimental.shard_map import shard_map
    from jax.sharding import Mesh, NamedSharding, PartitionSpec
    from concourse import bass2jax, mybir

    nc = _build()
    x2 = np.asarray(inputs["x2"], np.float32)
    args = {k: np.asarray(inputs[k], np.float32)
            for k in ("W1a", "g1a", "b1a", "W2a", "W1b", "g1b", "b1b", "W2b")}
    in_maps = _make_in_maps(x2, args["W1a"], args["g1a"], args["b1a"],
                            args["W2a"], args["W1b"], args["g1b"],
                            args["b1b"], args["W2b"])

    bass2jax.install_neuronx_cc_hook()
    partition_name = (nc.partition_id_tensor.name
                      if nc.partition_id_tensor else None)
    in_names, out_names, out_avals, zero_outs = [], [], [], []
    for alloc in nc.m.functions[0].allocations:
        if not isinstance(alloc, mybir.MemoryLocationSet):
            continue
        name = alloc.memorylocations[0].name
        if alloc.kind == "ExternalInput":
            if name != partition_name:
                in_names.append(name)
        elif alloc.kind == "ExternalOutput":
            shape = tuple(alloc.tensor_shape)
            dtype = mybir.dt.np(alloc.dtype)
            out_names.append(name)
            out_avals.append(jax.core.ShapedArray(shape, dtype))
            zero_outs.append(np.zeros(shape, dtype))
    n_params = len(in_names)
    all_in_names = list(in_names) + list(out_names)
    if partition_name is not None:
        all_in_names.append(partition_name)

    xi = in_names.index("x")

    def _body(*a):
        operands = list(a)
        if partition_name is not None:
            operands.append(bass2jax.partition_id_tensor())
        for _ in range(chain):
            outs = bass2jax._bass_exec_p.bind(
                *operands,
                out_avals=tuple(out_avals),
                in_names=tuple(all_in_names),
                out_names=tuple(out_names),
                lowering_input_output_aliases=(),
                sim_require_finite=True,
                sim_require_nnan=True,
                nc=nc,
            )
            operands[xi] = outs[0]
        return tuple(outs)

    devices = jax.devices()[:NCORES]
    mesh = Mesh(np.asarray(devices), ("core",))
    spec = PartitionSpec("core")
    n_outs = len(out_names)
    fn = jax.jit(
        shard_map(_body, mesh=mesh,
                  in_specs=(spec,) * (n_params + n_outs),
                  out_specs=(spec,) * n_outs, check_rep=False),
        keep_unused=True,
    )
    sharding = NamedSharding(mesh, spec)
    concat_in = [
        jax.device_put(
            np.concatenate([np.asarray(in_maps[c][nm]) for c in range(NCORES)],
                           axis=0), sharding)
        for nm in in_names
    ]
    concat_zero = [
        jax.device_put(np.zeros((NCORES * z.shape[0], *z.shape[1:]), z.dtype),
                       sharding)
        for z in zero_outs
    ]
    for _ in range(warmup):
        r = fn(*concat_in, *concat_zero)
        jax.block_until_ready(r)
    times = []
    for _ in range(iters):
        t0 = time.perf_counter()
        r = fn(*concat_in, *concat_zero)
        jax.block_until_ready(r)
        times.append(time.perf_counter() - t0)
    out = np.asarray(r[0]).reshape(NCORES, ROWS, C).reshape(N, C)
    return times, out


def kernel(**inputs):
    x2 = np.asarray(inputs["x2"], np.float32)
    npoint = np.asarray(inputs["npoint"])
    if (x2.shape != (N, C) or npoint.shape != (B,)
            or not np.all(npoint == SEG)):
        return _numpy_fallback(
            x2, npoint,
            *[np.asarray(inputs[k], np.float32)
              for k in ("W1a", "g1a", "b1a", "W2a", "W1b", "g1b", "b1b", "W2b")],
        ).astype(np.float32)
    out, _ = run_on_device(inputs)
    return out
